# revision 16
# baseline (speedup 1.0000x reference)
"""Cross-image contrastive loss on 8 TRN2 NeuronCores.

Strategy (row-parallel over the N=4096 pixel dim, 512 rows per core):
  - Host does all O(N) prep (label one-hots, mask folding, weights w,
    diag t1) and the final O(N) epilogue (Z -> logZ -> weighted sum).
  - Device does the O(N^2) work as 16 PSUM groups of [128, 2048] raw
    logits via fp8(e4m3) DoubleRow matmuls (2 cols/cycle):
      S1 = Fi^T Fii (K=64 packed 32x2), S2 augmented (K=84 packed 42x2,
      label mask folded into the contraction).
  - exp + row-sum split across engines by PATTERN:
      'A' groups: Act engine exact Exp(scale=1/tau) with accum_out.
      'D' groups: DVE computes s = l*A1 + B1 -> int16 (Schraudolph bits:
      bitcast-as-bf16 is ~exp(l/TAU)); one DMA block-transpose turns the
      dump into 16 [128(q), 128(p)] tiles; 16 chained PE matmuls with a
      bf16 ones vector accumulate them into a [1, 128] per-pixel sum.
  - Outputs: acc [128,16] (A-group sums) + zrow [1, 1024] (D-group
    sums); host combines, takes log, applies weights, sums partials.
"""

import math
import sys

import numpy as np

sys.path.insert(0, "/opt/trn_rl_repo")

import ml_dtypes

TAU = 0.07
EPS = 1e-4
L = 19
D = 64
N = 4096
NCORES = 8
P = N // NCORES  # 512 rows per core
KA = D + L + 1  # 84 augmented contraction for S2
CMASK = 4.0  # fp8-exact mask magnitude; CMASK/TAU ~ 57 in the exponent
PB = P // 128  # 4 partition blocks per core

# Schraudolph constants: bf16 bits v ~ round(l*A1 + B1) give
# 2^((v-16256)/128) ~ exp(l/TAU).  B1 centered so E[approx/exact] ~ 1.
A1 = 128.0 / (TAU * math.log(2.0))
B1 = 16256.0 - 7.37

# consumer per group: 'A' = Act exact exp, 'D' = DVE Schraudolph + PE reduce
PATTERN = "ADADADADADADADAA"

# packed input column layout (fp8, partitions 0:42)
C_LHS1 = 0  # [32, 1024]  Fi DoubleRow-packed, m-major
C_LHS2 = 1024  # [42, 1024]  augmented lhs, m-major
C_RHS1 = 2048  # [32, 8192]  Fii DoubleRow-packed, n-major
C_RHS2 = 10240  # [42, 8192]  augmented rhs, n-major
C_TOT = 18432

_compiled = None


def _build():
    from concourse import bacc, mybir, tile

    f32 = mybir.dt.float32
    bf16 = mybir.dt.bfloat16
    i16 = mybir.dt.int16
    f8 = mybir.dt.float8e4
    Exp = mybir.ActivationFunctionType.Exp
    add = mybir.AluOpType.add
    mult = mybir.AluOpType.mult
    DR = mybir.MatmulPerfMode.DoubleRow

    nc = bacc.Bacc("TRN2", target_bir_lowering=False, debug=False)

    inp_d = nc.dram_tensor("inp", (42, C_TOT), f8, kind="ExternalInput")
    acc_d = nc.dram_tensor("acc", (128, 16), f32, kind="ExternalOutput")
    zrow_d = nc.dram_tensor("zrow", (1, 1024), f32, kind="ExternalOutput")

    with tile.TileContext(nc) as tc:
        with (
            tc.tile_pool(name="res", bufs=1) as res,
            tc.tile_pool(name="dmp", bufs=2) as dmp,
            tc.tile_pool(name="ps", bufs=2, space="PSUM") as psp,
        ):
            nc.scalar.add_instruction(
                mybir.InstLoadActFuncSet(
                    name=nc.get_next_instruction_name(),
                    act_func_set_id=6,  # natural_log_exp_and_others
                    ins=[],
                    outs=[],
                )
            )

            # ---- resident SBUF tensors ----
            inp_sb = res.tile([42, C_TOT], f8, tag="inp")
            acc = res.tile([128, 16], f32, tag="acc")
            zrow = res.tile([1, 1024], f32, tag="zrow")
            ones = res.tile([128, 1], bf16, tag="ones")
            adump = res.tile([128, 2048], bf16, tag="adump")
            nc.vector.memset(acc[:], 0.0)
            nc.vector.memset(zrow[:], 0.0)
            nc.vector.memset(ones[:], 1.0)

            # two input DMAs: hot (lhs + rhs1) first, then rhs2
            nc.sync.dma_start(inp_sb[:, 0:C_RHS2], inp_d[:, 0:C_RHS2])
            nc.sync.dma_start(inp_sb[:, C_RHS2:C_TOT], inp_d[:, C_RHS2:C_TOT])

            # DoubleRow operand views, K-half blocks contiguous:
            # lhs col = b*256 + i*128 + m; rhs col = c*1024 + i*512 + n
            def lhs_view(s, b):
                base = C_LHS1 if s == 0 else C_LHS2
                kp = 32 if s == 0 else 42
                sl = inp_sb[0:kp, base + b * 256 : base + (b + 1) * 256]
                return sl.rearrange("p (i m) -> p i m", i=2)

            def rhs_view(s, c):
                base = C_RHS1 if s == 0 else C_RHS2
                kp = 32 if s == 0 else 42
                sl = inp_sb[0:kp, base + c * 1024 : base + (c + 1) * 1024]
                return sl.rearrange("p (i n) -> p i n", i=2)

            # ---- 16 groups: matmul -> exp+row-sum ----
            # group g = (s, b, h): s in {0:S1, 1:S2}, p-block b, col-half h
            didx = 0
            for g in range(16):
                s, b, h = g >> 3, (g >> 1) & 3, g & 1
                col = b * 4 + s * 2 + h  # acc col: p-block-major
                ps = psp.tile([128, 2048], f32, tag="mm")
                for j in range(4):
                    nc.tensor.matmul(
                        ps[:, j * 512 : (j + 1) * 512],
                        lhs_view(s, b),
                        rhs_view(s, h * 4 + j),
                        start=True,
                        stop=True,
                        perf_mode=DR,
                    )
                if PATTERN[g] == "A":
                    nc.scalar.activation(
                        adump[:],
                        ps[:],
                        Exp,
                        bias=0.0,
                        scale=1.0 / TAU,
                        accum_out=acc[:, col : col + 1],
                    )
                else:
                    ddump = dmp.tile([128, 2048], i16, tag="ddump")
                    nc.vector.tensor_scalar(ddump[:], ps[:], A1, B1, mult, add)
                    # block transpose: dT[q, k*128+p] = ddump[p, k*128+q]
                    dT = dmp.tile([128, 2048], i16, tag="dT")
                    nc.sync.dma_start_transpose(
                        dT[:].rearrange("q (k p) -> q k p", k=16),
                        ddump[:],
                    )
                    # per-pixel sums via 16 chained ones-matmuls into a
                    # spare strip of this group's own psum tile (only
                    # overlaps the j=3 region of the group 2 slots later)
                    zp = ps[0:1, 1536 : 1536 + 128]
                    dTb = dT[:].bitcast(bf16)
                    for k in range(16):
                        nc.tensor.matmul(
                            zp,
                            ones[:],
                            dTb[:, k * 128 : (k + 1) * 128],
                            start=(k == 0),
                            stop=(k == 15),
                        )
                    nc.vector.tensor_copy(
                        zrow[0:1, didx * 128 : (didx + 1) * 128], zp
                    )
                    didx += 1

            nc.scalar.dma_start(acc_d[:], acc[:])
            nc.sync.dma_start(zrow_d[:], zrow[:])

    nc.compile()
    return nc


def _prep(features_i, features_ii, features_jj, i, ii, jj):
    """Host-side prep: per-core device inputs + host epilogue arrays."""
    f8 = ml_dtypes.float8_e4m3fn
    Fi = features_i.reshape(D, N).astype(np.float32)
    Fii = features_ii.reshape(D, N).astype(np.float32)
    Fjj = features_jj.reshape(D, N).astype(np.float32)
    lab = i.reshape(-1)
    ii_f = ii.reshape(-1)
    jj_f = jj.reshape(-1)

    lids = np.arange(L, dtype=np.int32)
    oh_jj = (jj_f[None, :] == lids[:, None]).astype(np.float32)  # [L, N]

    aug_r = np.zeros((KA, N), np.float32)
    aug_r[0:D] = Fjj
    aug_r[D : D + L] = CMASK * oh_jj
    aug_r[D + L] = -CMASK

    def pack_dr(M, kp, blk):
        # [2*kp, X] -> [kp, 2X]; col = c*2*blk + i*blk + n (K-half blocks
        # contiguous per blk-sized column chunk)
        assert M.shape[0] == 2 * kp
        X = M.shape[1]
        out = np.zeros((kp, 2 * X), M.dtype)
        for c in range(X // blk):
            cs = slice(c * blk, (c + 1) * blk)
            out[:, 2 * c * blk : 2 * c * blk + blk] = M[:kp, cs]
            out[:, 2 * c * blk + blk : 2 * (c + 1) * blk] = M[kp:, cs]
        return out

    cnt_ii = np.bincount(ii_f, minlength=L).astype(np.float32)
    cnt_jj = np.bincount(jj_f, minlength=L).astype(np.float32)
    wl = cnt_ii / (cnt_ii + cnt_jj + EPS)

    rhs1_p = np.zeros((42, 2 * N), np.float32)
    rhs1_p[0:32] = pack_dr(Fii, 32, 512)
    rhs2_p = pack_dr(aug_r, 42, 512)  # [42, 8192]

    in_maps, host = [], []
    for c in range(NCORES):
        sel = slice(c * P, (c + 1) * P)
        lab_c = lab[sel]
        Fic = Fi[:, sel]

        aug_l = np.zeros((KA, P), np.float32)
        aug_l[0:D] = Fic
        aug_l[D : D + L] = (lab_c[None, :] == lids[:, None]).astype(np.float32)
        aug_l[D + L] = 1.0

        inp = np.zeros((42, C_TOT), np.float32)
        inp[0:32, C_LHS1 : C_LHS1 + 1024] = pack_dr(Fic, 32, 128)
        inp[:, C_LHS2 : C_LHS2 + 1024] = pack_dr(aug_l, 42, 128)
        inp[:, C_RHS1 : C_RHS1 + 2 * N] = rhs1_p
        inp[:, C_RHS2 : C_RHS2 + 2 * N] = rhs2_p
        in_maps.append({"inp": inp.astype(f8)})

        w = -wl[lab_c] / N  # [P]
        t1 = (Fic * (Fii[:, sel] + Fjj[:, sel])).sum(0) / TAU  # [P]
        host.append({"w": w, "t1": t1})
    return in_maps, host


def _combine(acc, zrow, w, t1):
    """Host epilogue for one core: Z -> logZ -> weighted sum."""
    Z = np.zeros(P, np.float64)
    didx = 0
    for g in range(16):
        s, b, h = g >> 3, (g >> 1) & 3, g & 1
        blk = slice(b * 128, (b + 1) * 128)
        if PATTERN[g] == "A":
            Z[blk] += acc[:, b * 4 + s * 2 + h].astype(np.float64)
        else:
            Z[blk] += zrow.reshape(-1)[didx * 128 : (didx + 1) * 128].astype(
                np.float64
            )
            didx += 1
    vals = w.astype(np.float64) * (t1.astype(np.float64) - 2.0 * np.log(Z + EPS))
    return vals.sum()


_LDW_PATCHED = False


def _enable_ldw_opt():
    """Flip walrus --enable-ldw-opt for this process (dedups back-to-back
    LDWEIGHTS of the same stationary operand)."""
    global _LDW_PATCHED
    if _LDW_PATCHED:
        return
    from concourse import bass_utils

    orig = bass_utils.run_command

    def patched(cmd, *a, **kw):
        if isinstance(cmd, list):
            cmd = [
                "--enable-ldw-opt=true" if c == "--enable-ldw-opt=false" else c
                for c in cmd
            ]
        return orig(cmd, *a, **kw)

    bass_utils.run_command = patched
    _LDW_PATCHED = True


def kernel(features_i, features_ii, features_jj, i, ii, jj):
    global _compiled
    from concourse import bass_utils

    if _compiled is None:
        _compiled = _build()
    in_maps, host = _prep(features_i, features_ii, features_jj, i, ii, jj)
    results = bass_utils.run_bass_kernel_spmd(
        _compiled, in_maps, core_ids=list(range(NCORES))
    )
    total = 0.0
    for c, r in enumerate(results.results):
        total += _combine(
            r["acc"], r["zrow"], host[c]["w"], host[c]["t1"]
        )
    return np.array(total, dtype=np.float32)
